# revision 22
# baseline (speedup 1.0000x reference)
"""Cyclic group-conv kernel for TRN2, 8 NeuronCores.

out[b, h, e] = sum_{g,i} input[b, g, i] * weights[inv_indices[h, g], i, e] + bias[e]

Shapes (hardcoded): B=256, G=24, I=512, E=512.

Sharding: 2-way split of B (128 each) x 4-way split of h (6 each) = 8 cores.
All per-core variation lives in the DATA (input slice, permuted weight copy);
the SPMD program is identical on every core:

    for j in 0..G-1:  load Wc[j] (a [512,512] weight matrix, per-core order)
        for i-chunk c in 0..3, local h in 0..5:
            psum[h] += inpT[:, c, Gtab[j][h], :].T @ Wc[j][c]

where Gtab[j][h_loc] (uniform across cores) and the per-core weight order
tau_c[j] are solved on the host from the runtime inv_indices table so that
    tau_c[j] == inv_indices[h_core(h_loc), Gtab[j][h_loc]]  for every h_loc.
For the cyclic table inv[h,g] = (g-h) % G this always has a solution.

Most matmuls run fp16 (full-rate on the PE at moving-dim 512).  K_FP8 of
the 24 j-steps run as e4m3 DoubleRow matmuls (2x PE rate), trading a small,
measured accuracy hit (norm rel err stays well under the 2e-2 gate) for
~2.6us of PE time per converted step.  fp8 operands are prescaled
(input/4, weights*4) so both land in e4m3's normal range; the product
scale is 1 so fp8 and fp16 steps share the same PSUM accumulation.
Output is stored fp16 (halves the out-DMA) and upcast on the host.
"""

import numpy as np

B, G, I, E = 256, 24, 512, 512
NB, NH = 2, 4  # batch-split x h-split = 8 cores
BL = B // NB  # 128 rows per core  -> matmul M dim
HL = G // NH  # 6 h values per core -> 6 PSUM banks
IC = I // 128  # 4 contraction chunks

K_FP8 = 5  # j-steps converted to fp8 DoubleRow
SX, SW = 0.25, 4.0  # fp8 prescale: input*SX, weights*SW, SX*SW == 1

_LAST_RESULTS = None  # stashed BassKernelResults for test harness introspection

GQ = 1  # input g's packed per DMA (1 => finest head gating)
NQ = G // GQ


def _fp8_js():
    # j=0 runs fp8: the head-critical DMA halves (fp8 inputs + weights),
    # so the PE starts ~2us earlier.  The rest sit mid-stream.
    return [0] + list(range(9, 9 + K_FP8 - 1))


def _use_order(gtab):
    """g indices in first-use order over the j/h loop."""
    order, seen = [], set()
    for j in range(G):
        for h in range(HL):
            g = int(gtab[j][h])
            if g not in seen:
                seen.add(g)
                order.append(g)
    return order


def _solve_schedule(inv: np.ndarray):
    """Uniform Gtab[j][h_loc] + per-h-group weight order tau[i_h][j]."""
    ginv = np.argsort(inv, axis=1)  # ginv[h, s] = g with inv[h, g] == s
    gtab = ginv[:HL, :].T.copy()  # [G, HL]: core-0 reference schedule
    taus = []
    for ih in range(NH):
        h_vals = ih * HL + np.arange(HL)
        # s required at step j for each local h
        s = inv[h_vals[None, :].repeat(G, 0), gtab]  # [G, HL]
        if not (s == s[:, :1]).all():
            raise ValueError(
                "inv_indices table does not admit a uniform SPMD schedule "
                "for h-group %d" % ih
            )
        tau = s[:, 0]
        if sorted(tau.tolist()) != list(range(G)):
            raise ValueError("tau is not a permutation for h-group %d" % ih)
        taus.append(tau)
    return gtab, taus


def _build_program(gtab):
    import concourse.bass as bass  # noqa: F401
    import concourse.tile as tile
    from concourse import bacc, mybir

    dt = mybir.dt
    nc = bacc.Bacc(
        "TRN2", target_bir_lowering=False, debug=False, num_devices=NB * NH
    )

    fp8_js = _fp8_js()
    fp8_idx = {j: i for i, j in enumerate(fp8_js)}
    g8 = sorted({int(gtab[j][h]) for j in fp8_js for h in range(HL)})
    g8pos = {g: i for i, g in enumerate(g8)}
    ng8 = len(g8)

    inp_d = nc.dram_tensor(
        "inp", [NQ, 128, GQ, IC, BL], dt.float16, kind="ExternalInput"
    )
    inp8_d = nc.dram_tensor(
        "inp8", [ng8, 128, IC, BL], dt.float8e4, kind="ExternalInput"
    )
    w_d = nc.dram_tensor(
        "w", [G - K_FP8, 128, IC, E], dt.float16, kind="ExternalInput"
    )
    w8_d = nc.dram_tensor(
        "w8", [K_FP8, 128, IC, E], dt.float8e4, kind="ExternalInput"
    )
    bias_d = nc.dram_tensor("bias", [128, E], dt.float32, kind="ExternalInput")
    out_d = nc.dram_tensor("out", [HL, BL, E], dt.float16, kind="ExternalOutput")

    # fp16 weight slot in w_d for each non-fp8 j (in j order)
    w16_idx = {}
    for j in range(G):
        if j not in fp8_idx:
            w16_idx[j] = len(w16_idx)

    # Host packs input group q at position r with g = use_order[q*GQ + r]
    # (first-use order so early matmuls' tiles land first).
    use_order = _use_order(gtab)
    pos_of_g = {g: divmod(i, GQ) for i, g in enumerate(use_order)}
    # first FP16 j needing group q -> emit its DMA just before that j
    # (fp8 j-steps read the separate fp8 copies, not these tiles)
    first_j_for_q = {}
    for j in range(G):
        if j in fp8_idx:
            continue
        for h in range(HL):
            q = pos_of_g[int(gtab[j][h])][0]
            if q not in first_j_for_q:
                first_j_for_q[q] = j
    q_emit_at = {}  # j -> list of q to emit before iteration j
    for q, j0 in first_j_for_q.items():
        q_emit_at.setdefault(j0, []).append(q)
    # fp8 g-tiles used by j=0 (emitted inline at j=0, head-critical order)
    g8_head = []
    if 0 in fp8_idx:
        for h in range(HL):
            g = int(gtab[0][h])
            if g not in g8_head:
                g8_head.append(g)
    g8_rest = [g for g in g8 if g not in g8_head]

    with tile.TileContext(nc) as tc:
        with (
            tc.tile_pool(name="inp", bufs=1) as ipool,
            tc.tile_pool(name="in8", bufs=1) as i8pool,
            tc.tile_pool(name="wp", bufs=4) as wpool,
            tc.tile_pool(name="w8p", bufs=1) as w8pool,
            tc.tile_pool(name="ps", bufs=1, space="PSUM") as pspool,
            tc.tile_pool(name="op", bufs=1) as opool,
        ):
            inp_t = [None] * NQ
            inp8_t = [None] * ng8
            w8_t = [None] * K_FP8
            psum = [
                pspool.tile([BL, E], dt.float32, tag=f"ps{h}", name=f"ps{h}")
                for h in range(HL)
            ]
            bias_t = None

            # PE pre-warm: dummy matmuls on a zeroed tile into a spare PSUM
            # bank while the first DMAs are in flight, so the HAM clock
            # gate opens (1.2 -> 2.4 GHz) before the real matmuls start.
            # Small memset ([128,128] fp32) so the first warm matmul issues
            # as early as possible after the NEFF preamble.
            warm_f32 = ipool.tile([128, 128], dt.float32, tag="warm")
            nc.gpsimd.memset(warm_f32[:], 0.0)
            warm_bf = warm_f32.bitcast(dt.float16)
            warm_ps = pspool.tile([128, 256], dt.float32, tag="warmps")
            for _ in range(16):
                nc.tensor.matmul(
                    warm_ps[:, :],
                    lhsT=warm_bf[:, :128],
                    rhs=warm_bf[:],
                    start=True,
                    stop=True,
                )

            def lhs(j, h, c):
                q, r = pos_of_g[int(gtab[j][h])]
                return inp_t[q][:, r, c, :]

            def emit_q(q):
                inp_t[q] = ipool.tile(
                    [128, GQ, IC, BL], dt.float16, tag=f"in{q}", name=f"in{q}"
                )
                nc.sync.dma_start(out=inp_t[q][:], in_=inp_d[q])

            def emit_i8(g):
                i = g8pos[g]
                inp8_t[i] = i8pool.tile(
                    [128, IC, BL], dt.float8e4, tag=f"i8_{i}", name=f"i8_{i}"
                )
                nc.sync.dma_start(out=inp8_t[i][:], in_=inp8_d[i])

            def alloc_w8(k):
                w8_t[k] = w8pool.tile(
                    [128, IC, E], dt.float8e4, tag=f"w8_{k}", name=f"w8_{k}"
                )

            # fp8 DMAs not needed at j=0 go on the sync queue at j=5..7,
            # AFTER the head-critical triggers but well before their ~50us
            # first use.  All K_FP8 weight tiles are live at once so no
            # fp8 step waits on a WAR of an earlier fp8 tile.
            rest = [("i", g) for g in g8_rest] + [
                ("w", k) for k in range(K_FP8) if _fp8_js()[k] != 0
            ]
            per_j = (len(rest) + 2) // 3

            for j in range(G):
                if 5 <= j <= 7:
                    for kind, v in rest[(j - 5) * per_j : (j - 4) * per_j]:
                        if kind == "i":
                            emit_i8(v)
                        else:
                            alloc_w8(v)
                            nc.sync.dma_start(out=w8_t[v][:], in_=w8_d[v])
                if j in fp8_idx:
                    # fp8 step: one [128, IC, E] e4m3 weight tile, DoubleRow
                    # matmuls consume c-chunk pairs at 2x PE rate.
                    k = fp8_idx[j]
                    if j == 0:
                        # Head: interleave the split weight halves with the
                        # six g-tiles so the first matmul gates on ~192KB.
                        alloc_w8(k)
                        nc.sync.dma_start(
                            out=w8_t[k][:, 0:2, :], in_=w8_d[k, :, 0:2]
                        )
                        for g in g8_head:
                            emit_i8(g)
                        nc.sync.dma_start(
                            out=w8_t[k][:, 2:4, :], in_=w8_d[k, :, 2:4]
                        )
                    wk = w8_t[k]
                    for q in q_emit_at.get(j, []):
                        emit_q(q)
                    for cp in (0, 2):
                        for h in range(HL):
                            g = int(gtab[j][h])
                            nc.tensor.matmul(
                                psum[h][:, :],
                                lhsT=inp8_t[g8pos[g]][:, cp : cp + 2, :],
                                rhs=wk[:, cp : cp + 2, :],
                                start=(j == 0 and cp == 0),
                                stop=False,
                                perf_mode=mybir.MatmulPerfMode.DoubleRow,
                            )
                    continue

                # Per-chunk weight tiles: compute gates on 128KB slices.
                w_c = []

                def emit_wc(c, j=j, w_c=w_c):
                    t = wpool.tile(
                        [128, E], dt.float16, tag=f"wc{c}", name=f"w{j}c{c}"
                    )
                    nc.sync.dma_start(out=t[:], in_=w_d[w16_idx[j], :, c])
                    w_c.append(t)

                if j == 0:
                    # j=0: interleave input groups and weight chunks so the
                    # first matmul gates on ~0.4MB of DMA.
                    q0 = q_emit_at.get(0, [])
                    emit_q(q0[0])
                    emit_wc(0)
                    for q in q0[1:]:
                        emit_q(q)
                    for c in range(1, IC):
                        emit_wc(c)
                else:
                    for q in q_emit_at.get(j, []):
                        emit_q(q)
                    for c in range(IC):
                        emit_wc(c)
                if j == 20:
                    bias_t = opool.tile([128, E], dt.float32, tag="bias")
                    nc.scalar.dma_start(out=bias_t[:], in_=bias_d[:])
                if j < G - 1:
                    for c in range(IC):
                        for h in range(HL):
                            nc.tensor.matmul(
                                psum[h][:, :],
                                lhsT=lhs(j, h, c),
                                rhs=w_c[c][:],
                                start=(j == 0 and c == 0),
                                stop=False,
                            )
                else:
                    # Last j: finish PSUM banks one h at a time so the
                    # bias-add + store of early h overlap the rest.  The
                    # final h's drain is the serial tail: split it into
                    # column halves on two engines with two DGE queues.
                    for h in range(HL):
                        for c in range(IC):
                            nc.tensor.matmul(
                                psum[h][:, :],
                                lhsT=lhs(j, h, c),
                                rhs=w_c[c][:],
                                start=False,
                                stop=(c == IC - 1),
                            )
                        o_t = opool.tile(
                            [BL, E], dt.float16, tag=f"o{h}", name=f"o{h}"
                        )
                        if h < HL - 1:
                            nc.vector.tensor_add(o_t[:], psum[h][:], bias_t[:])
                            nc.scalar.dma_start(out=out_d[h], in_=o_t[:])
                        else:
                            eh = E // 2
                            nc.vector.tensor_add(
                                o_t[:, :eh], psum[h][:, :eh], bias_t[:, :eh]
                            )
                            nc.scalar.dma_start(
                                out=out_d[h, :, :eh], in_=o_t[:, :eh]
                            )
                            nc.vector.tensor_add(
                                o_t[:, eh:], psum[h][:, eh:], bias_t[:, eh:]
                            )
                            nc.sync.dma_start(
                                out=out_d[h, :, eh:], in_=o_t[:, eh:]
                            )

    nc.compile()
    return nc


def kernel(input, weights, bias, inv_indices):
    global _LAST_RESULTS
    import ml_dtypes
    from concourse.bass_utils import run_bass_kernel_spmd

    input = np.asarray(input, dtype=np.float32)
    weights = np.asarray(weights, dtype=np.float32)
    bias = np.asarray(bias, dtype=np.float32)
    inv = np.asarray(inv_indices).astype(np.int64)

    gtab, taus = _solve_schedule(inv)
    nc = _build_program(gtab)

    fp8_js = _fp8_js()
    g8 = sorted({int(gtab[j][h]) for j in fp8_js for h in range(HL)})
    w16_js = [j for j in range(G) if j not in fp8_js]

    # Per-core input: groups [NQ, 128, GQ, IC, BL], group q slot r holds
    # g = use_order[q*GQ+r] as inpT[g][p, c, b] = input[b0+b, g, c*128+p]
    use_order = _use_order(gtab)
    inp_arrs, inp8_arrs = [], []
    for ib in range(NB):
        sl = input[ib * BL : (ib + 1) * BL]  # [BL, G, I]
        t = sl.transpose(1, 2, 0).reshape(G, IC, 128, BL).transpose(0, 2, 1, 3)
        packed = np.empty((NQ, 128, GQ, IC, BL), np.float16)
        for i, g in enumerate(use_order):
            q, r = divmod(i, GQ)
            packed[q, :, r] = t[g]
        inp_arrs.append(packed)
        p8 = np.ascontiguousarray(t[g8] * SX).astype(ml_dtypes.float8_e4m3)
        inp8_arrs.append(p8)  # [ng8, 128, IC, BL]
    # Per-h-group weights, reordered: fp16 [G-K, 128, IC, E], fp8 [K, ...]
    w_arrs, w8_arrs = [], []
    for tau in taus:
        wt = weights[tau].reshape(G, IC, 128, E).transpose(0, 2, 1, 3)
        w_arrs.append(
            np.ascontiguousarray(wt[w16_js]).astype(np.float16)
        )
        w8_arrs.append(
            np.ascontiguousarray(wt[fp8_js] * SW).astype(ml_dtypes.float8_e4m3)
        )
    bias_rep = np.ascontiguousarray(np.broadcast_to(bias, (128, E)))

    core_ids = list(range(NB * NH))
    in_maps = []
    for k in core_ids:
        ib, ih = k % NB, k // NB
        in_maps.append(
            {
                "inp": inp_arrs[ib],
                "inp8": inp8_arrs[ib],
                "w": w_arrs[ih],
                "w8": w8_arrs[ih],
                "bias": bias_rep,
            }
        )

    res = run_bass_kernel_spmd(nc, in_maps, core_ids)
    _LAST_RESULTS = res

    full = np.empty((B, G, E), dtype=np.float32)
    for k in core_ids:
        ib, ih = k % NB, k // NB
        ock = res.results[k]["out"].astype(np.float32)  # [HL, BL, E] fp16
        full[ib * BL : (ib + 1) * BL, ih * HL : (ih + 1) * HL] = ock.transpose(
            1, 0, 2
        )
    return full
